# revision 7
# baseline (speedup 1.0000x reference)
"""Bass/Trainium2 kernel for nn_BilinearMixture (gnn_message_passing).

out = softmax(einsum('ed,kd,ed->ek', u_feat[u_idx], basis_weights, v_feat[v_idx])
              @ weights_scalars + user_bias[u_idx] + item_bias[v_idx])

Strategy (8 NeuronCores, SPMD, edges sharded across cores):
  * Fold the K x D x C basis projection into one M = basis_weights.T @ weights_scalars
    [D, C] matrix (computed on device), so per edge: logits = (u .* v) @ M.
  * Biases are folded into the gather via row augmentation:
      u' = [u, user_bias, 1s, 0...],  v' = [v, 1s, item_bias, 0...],  M' = [M; I; I; 0]
  * Tables are fp16, 128-element (256 B) rows (dma_gather packets must be a
    multiple of 256 B). Edge rows are fetched edge-major with dma_gather in
    784-slot sub-gathers (NSG=10 per side per tile): each SWDGE ring entry
    drains with its own ~128-descriptor in-flight window and entries drain
    concurrently, so ~5 small entries resident per queue keep ~640 descs in
    flight per queue and the DMA engines saturated. 16 trailing idx-0 slots
    per sub-gather keep the trailing-negative-skip rule from dropping real
    (signed-offset) slots.
  * int16 gather indices address a 65536-row window via signed offsets from a
    base placed 32768 rows into the table; u needs 2 windows, v 1; edges
    bucket per-core by u-window only.
  * PE transposes flip uv chunks to feature-major; the projection matmul is
    sliced to the 74 live partitions so no memset of uvt is needed. PSUM->SBUF
    logits copies and Exp run on the Scalar engine; PE transposes flip logits
    back to edge-major for the softmax. Output fp16, widened host-side.
"""

import numpy as np

NU, NI, D, E0, K, C = 100000, 50000, 64, 2000000, 3, 5
CORES = 8
EL = 128              # fp16 elements per padded table row (256 B)
CHUNK = 512           # edges per projection matmul (PSUM bank: 512 f32)
NCH = 3               # chunks per PSUM bank (PE col positions {0,32,64})
QE = NCH * CHUNK      # edges per PSUM bank group (1536)
NQT = 5               # bank groups per tile
T = NQT * QE          # edges per tile (7680)
NSG = 10              # sub-gathers per side per tile
SGR = T // NSG        # real slots per sub-gather (1280)
PAD = 16              # trailing idx-0 slots per sub-gather
SG = SGR + PAD        # gather slots per sub-gather (1296)
SGC = (SG + 127) // 128   # chunks per sub-gather dest (11)
RC = SGR // 128       # real chunks per sub-gather (10)
SCRATCH = 65536       # SWDGE ring: 4096 descs/queue = ~3 sub-gathers deep
WIN = 65536           # rows per gather window (signed int16 + base)
BASE = 32768          # base row offset within a window
USEG = 2              # ceil(NU / WIN)
UROWS = USEG * WIN
VROWS = WIN

_PROG_CACHE = {}


def _build_program(schedule):
    import concourse.bacc as bacc
    import concourse.mybir as mybir
    import concourse.tile as tile
    from concourse.masks import make_identity

    NT = len(schedule)
    f16, f32, i16 = mybir.dt.float16, mybir.dt.float32, mybir.dt.int16

    nc = bacc.Bacc("TRN2", target_bir_lowering=False, debug=False, num_devices=CORES,
                   num_swdge_queues=4, dynamic_dma_scratch_size=SCRATCH)
    u16 = nc.dram_tensor("u16", [UROWS, EL], f16, kind="ExternalInput")
    v16 = nc.dram_tensor("v16", [VROWS, EL], f16, kind="ExternalInput")
    uidx = nc.dram_tensor("uidx", [NT, 128, NSG * (SG // 16)], i16, kind="ExternalInput")
    vidx = nc.dram_tensor("vidx", [NT, 128, NSG * (SG // 16)], i16, kind="ExternalInput")
    bw = nc.dram_tensor("bw", [K, D], f32, kind="ExternalInput")
    ws = nc.dram_tensor("ws", [K, C], f32, kind="ExternalInput")
    eye = nc.dram_tensor("eye", [2 * C, C], f16, kind="ExternalInput")
    out = nc.dram_tensor("out", [NT * NQT, 128, 4 * NCH * C], f16, kind="ExternalOutput")

    DV = D + 2 * C            # live feature rows (74)

    with tile.TileContext(nc) as tc:
        with (
            tc.tile_pool(name="const", bufs=1) as cpool,
            tc.tile_pool(name="idx", bufs=3) as ipool,
            tc.tile_pool(name="ut", bufs=2) as utpool,
            tc.tile_pool(name="vt", bufs=2) as vtpool,
            tc.tile_pool(name="uv", bufs=2) as uvpool,
            tc.tile_pool(name="uvt", bufs=2) as uvtpool,
            tc.tile_pool(name="psU", bufs=2, space="PSUM") as psupool,
            tc.tile_pool(name="sba", bufs=3) as sbapool,
            tc.tile_pool(name="ex", bufs=3) as expool,
            tc.tile_pool(name="sm", bufs=3) as smpool,
            tc.tile_pool(name="ob", bufs=3) as obpool,
            tc.tile_pool(name="psA", bufs=2, space="PSUM") as psapool,
            tc.tile_pool(name="psT", bufs=2, space="PSUM") as pstpool,
        ):
            # ---- prologue: M' = [W.T @ ws ; I5 ; I5 ; 0] as fp16 [128, C] ----
            w_sb = cpool.tile([K, D], f32)
            nc.sync.dma_start(w_sb[:], bw[:])
            ws_sb = cpool.tile([K, C], f32)
            nc.sync.dma_start(ws_sb[:], ws[:])
            mpsum = psapool.tile([D, C], f32)
            nc.tensor.matmul(out=mpsum[:], lhsT=w_sb[:], rhs=ws_sb[:], start=True, stop=True)
            m16 = cpool.tile([128, C], f16)
            nc.gpsimd.memset(m16[:], 0)
            nc.scalar.activation(m16[0:D, :], mpsum[:],
                                 mybir.ActivationFunctionType.Copy)
            nc.sync.dma_start(m16[D:D + 2 * C, :], eye[:])
            ident16 = cpool.tile([128, 128], f16)
            make_identity(nc, ident16[:])

            # ---- main loop over tiles ----
            for t, su in enumerate(schedule):
                iu = ipool.tile([128, NSG * (SG // 16)], i16, tag="iu")
                nc.sync.dma_start(iu[:], uidx[t])
                iv = ipool.tile([128, NSG * (SG // 16)], i16, tag="iv")
                nc.sync.dma_start(iv[:], vidx[t])

                ut = utpool.tile([128, NSG * SGC * EL], f16, tag="ut")
                vt = vtpool.tile([128, NSG * SGC * EL], f16, tag="vt")
                for g in range(NSG):
                    nc.gpsimd.dma_gather(
                        ut[:, g * SGC * EL:(g + 1) * SGC * EL]
                            .rearrange("p (c e) -> p c e", e=EL),
                        u16[su * WIN + BASE: su * WIN + 2 * BASE, :],
                        iu[:, g * (SG // 16):(g + 1) * (SG // 16)],
                        SG, SG, EL, elem_step=EL, transpose=False,
                        single_packet=False, queue_num=(2 * g) % 4,
                    )
                    nc.gpsimd.dma_gather(
                        vt[:, g * SGC * EL:(g + 1) * SGC * EL]
                            .rearrange("p (c e) -> p c e", e=EL),
                        v16[BASE: 2 * BASE, :],
                        iv[:, g * (SG // 16):(g + 1) * (SG // 16)],
                        SG, SG, EL, elem_step=EL, transpose=False,
                        single_packet=False, queue_num=(2 * g + 1) % 4,
                    )

                # real slot s = g*SGR + c*128 + p  ->  ut[p, g*SGC+c chunk]
                uv = uvpool.tile([128, T], f16, tag="uv")
                uview = ut[:].rearrange("p (g c e) -> p g c e", g=NSG, c=SGC)[:, :, 0:RC, :]
                vview = vt[:].rearrange("p (g c e) -> p g c e", g=NSG, c=SGC)[:, :, 0:RC, :]
                nc.vector.tensor_mul(
                    out=uv[:].rearrange("p (g c e) -> p g c e", g=NSG, c=RC),
                    in0=uview, in1=vview)
                # PE block transposes: uvt[d, b*128+e] = uv[e, b*128+d]
                uvt = uvtpool.tile([128, T], f16, tag="uvt")
                for h in range(T // CHUNK):
                    psu = psupool.tile([128, CHUNK], f16, tag="psu")
                    for k in range(4):
                        b = h * 4 + k
                        nc.tensor.transpose(
                            out=psu[0:DV, k * 128:(k + 1) * 128],
                            in_=uv[:, b * 128:b * 128 + DV],
                            identity=ident16[:],
                        )
                    nc.vector.tensor_copy(
                        out=uvt[0:DV, h * CHUNK:(h + 1) * CHUNK], in_=psu[0:DV, :])

                for q in range(NQT):
                    psa = psapool.tile([128, CHUNK], f32, tag="psa")
                    for cpos in range(NCH):
                        e0 = q * QE + cpos * CHUNK
                        nc.tensor.matmul(
                            out=psa[32 * cpos:32 * cpos + C, :],
                            lhsT=m16[0:DV, :],
                            rhs=uvt[0:DV, e0:e0 + CHUNK],
                            start=True, stop=True,
                        )
                    sba = sbapool.tile([128, CHUNK], f16, tag="sba")
                    nc.scalar.activation(sba[:], psa[:],
                                         mybir.ActivationFunctionType.Copy)
                    # transpose to edge-major: col w*128+32c+j -> [p, (w,c,j)]
                    pst = pstpool.tile([128, CHUNK], f16, tag="pst")
                    NV = 64 + C
                    for w in range(4):
                        nc.tensor.transpose(
                            out=pst[:, w * 128:w * 128 + NV],
                            in_=sba[0:NV, w * 128:(w + 1) * 128],
                            identity=ident16[0:NV, 0:NV],
                        )
                    ex = expool.tile([128, CHUNK], f16, tag="ex")
                    nc.scalar.activation(ex[:], pst[:],
                                         mybir.ActivationFunctionType.Exp)
                    exv = ex[:].rearrange("p (w c j) -> p w c j", c=4, j=32)[:, :, 0:NCH, 0:C]
                    den = smpool.tile([128, 4 * NCH], f32, tag="den")
                    nc.vector.tensor_reduce(
                        out=den[:].rearrange("p (w c) -> p w c", c=NCH), in_=exv,
                        axis=mybir.AxisListType.X, op=mybir.AluOpType.add,
                    )
                    rec = smpool.tile([128, 4 * NCH], f32, tag="rec")
                    nc.vector.reciprocal(rec[:], den[:])
                    ob = obpool.tile([128, 4 * NCH * C], f16, tag="ob")
                    nc.vector.tensor_mul(
                        out=ob[:].rearrange("p (w c j) -> p w c j", c=NCH, j=C),
                        in0=exv,
                        in1=rec[:].rearrange("p (w c o) -> p w c o", c=NCH, o=1)
                               .to_broadcast([128, 4, NCH, C]),
                    )
                    nc.sync.dma_start(out[t * NQT + q, :, :], ob[:])

    nc.compile()
    return nc


def _get_program(schedule):
    key = tuple(schedule)
    if key not in _PROG_CACHE:
        _PROG_CACHE[key] = _build_program(key)
    return _PROG_CACHE[key]


def _wrap_idx(loc):
    """[NT, NSG, SG] int16 -> [NT, 128, NSG*(SG//16)]: within sub-gather g,
    slot i = j*16 + q at partition q (replicated x8), free position j."""
    NT = loc.shape[0]
    w = loc.reshape(NT, NSG, SG // 16, 16).transpose(0, 1, 3, 2)  # [NT,NSG,16,S]
    w = np.broadcast_to(w[:, :, None], (NT, NSG, 8, 16, SG // 16))
    w = w.reshape(NT, NSG, 128, SG // 16).transpose(0, 2, 1, 3)   # [NT,128,NSG,S]
    return np.ascontiguousarray(w.reshape(NT, 128, NSG * (SG // 16)))


def _prepare(u_feat, v_feat, u_indices, v_indices, basis_weights,
             weights_scalars, user_bias, item_bias):
    u_feat = np.asarray(u_feat, dtype=np.float32)
    v_feat = np.asarray(v_feat, dtype=np.float32)
    u_indices = np.asarray(u_indices, dtype=np.int32)
    v_indices = np.asarray(v_indices, dtype=np.int32)
    basis_weights = np.asarray(basis_weights, dtype=np.float32)
    weights_scalars = np.asarray(weights_scalars, dtype=np.float32)
    user_bias = np.asarray(user_bias, dtype=np.float32)
    item_bias = np.asarray(item_bias, dtype=np.float32)

    E = u_indices.shape[0]
    assert E % CORES == 0
    epc = E // CORES

    # ---- augmented fp16 tables (features + bias folding), 256 B rows ----
    u16 = np.zeros((UROWS, EL), np.float16)
    u16[:NU, :D] = u_feat.astype(np.float16)
    u16[:NU, D:D + C] = user_bias.astype(np.float16)
    u16[:NU, D + C:D + 2 * C] = np.float16(1.0)
    v16 = np.zeros((VROWS, EL), np.float16)
    v16[:NI, :D] = v_feat.astype(np.float16)
    v16[:NI, D:D + C] = np.float16(1.0)
    v16[:NI, D + C:D + 2 * C] = item_bias.astype(np.float16)
    eye = np.zeros((2 * C, C), np.float16)
    eye[:C] = np.eye(C, dtype=np.float16)
    eye[C:] = np.eye(C, dtype=np.float16)

    # ---- bucket edges per core by u-window only ----
    keys = (u_indices // WIN).astype(np.int64)
    orders, counts = [], np.zeros((CORES, USEG), np.int64)
    for c in range(CORES):
        kc = keys[c * epc:(c + 1) * epc]
        orders.append(np.argsort(kc, kind="stable"))
        counts[c] = np.bincount(kc, minlength=USEG)
    tiles_b = -(-counts.max(axis=0) // T)
    schedule = []
    for b in range(USEG):
        schedule += [b] * int(tiles_b[b])
    NT = len(schedule)
    ETOT = NT * T
    cap_base = np.concatenate([[0], np.cumsum(tiles_b * T)])

    in_maps, place = [], []
    for c in range(CORES):
        o = orders[c]
        ug = u_indices[c * epc:(c + 1) * epc]
        vg = v_indices[c * epc:(c + 1) * epc]
        u_flat = np.zeros(ETOT, np.int16)
        v_flat = np.zeros(ETOT, np.int16)
        pos = np.empty(epc, np.int64)
        start = 0
        for b in range(USEG):
            n = int(counts[c, b])
            sel = o[start:start + n]
            pb = cap_base[b] + np.arange(n)
            u_flat[pb] = (ug[sel] - b * WIN - BASE).astype(np.int16)
            v_flat[pb] = (vg[sel] - BASE).astype(np.int16)
            pos[start:start + n] = pb
            start += n
        # [NT, NSG, SG] with the trailing PAD slots at idx 0 (row BASE)
        u_loc = np.zeros((NT, NSG, SG), np.int16)
        v_loc = np.zeros((NT, NSG, SG), np.int16)
        u_loc[:, :, 0:SGR] = u_flat.reshape(NT, NSG, SGR)
        v_loc[:, :, 0:SGR] = v_flat.reshape(NT, NSG, SGR)
        in_maps.append({
            "u16": u16, "v16": v16,
            "uidx": _wrap_idx(u_loc),
            "vidx": _wrap_idx(v_loc),
            "bw": basis_weights, "ws": weights_scalars, "eye": eye,
        })
        place.append((pos, o))

    return schedule, in_maps, place, E, epc


def _unshard(results, place, E, epc):
    out = np.empty((E, C), np.float32)
    for c in range(CORES):
        oc = results[c]["out"]  # [NT*NQT, 128, 4*NCH*C] fp16
        NQ = oc.shape[0]
        # arr[q, p, w, c, j] -> edge q*QE + c*512 + w*128 + p
        arr = oc.reshape(NQ, 128, 4, NCH, C).transpose(0, 3, 2, 1, 4)
        flat = np.ascontiguousarray(arr).reshape(NQ * QE, C).astype(np.float32)
        pos, o = place[c]
        out[c * epc + o] = flat[pos]
    return out


def kernel(u_feat, v_feat, u_indices, v_indices, basis_weights,
           weights_scalars, user_bias, item_bias):
    import os
    # The bass kernel executes through the axon PJRT backend; don't let a
    # CPU-pinned JAX_PLATFORMS hide the NeuronCore devices.
    if os.environ.get("JAX_PLATFORMS") and "axon" not in os.environ["JAX_PLATFORMS"]:
        os.environ["JAX_PLATFORMS"] = ""
    from concourse.bass_utils import run_bass_kernel_spmd

    schedule, in_maps, place, E, epc = _prepare(
        u_feat, v_feat, u_indices, v_indices, basis_weights,
        weights_scalars, user_bias, item_bias)
    nc = _get_program(tuple(schedule))
    res = run_bass_kernel_spmd(nc, in_maps, core_ids=list(range(CORES)))
    global LAST_RESULT
    LAST_RESULT = res
    return _unshard([r for r in res.results], place, E, epc)
